# revision 50
# baseline (speedup 1.0000x reference)
"""Blocksparse conv2d (3x3, stride 1, pad 1) on 8 Trainium2 NeuronCores.

Strategy
--------
Data-parallel over batch: 16 images -> 2 per core, identical SPMD program.

The mask zeroes whole 32x32 (cout, cin) channel blocks; the host inspects
the runtime mask and keeps only the input-channel blocks that survive
(seed-42 mask: channels 64..127, K_used=64). Host-side prep (all cheap):
  - w*mask, slice to used channels, transpose to the PE lhsT layout
    [K_used, 9, COUT], replicate per partition half, cast to bf16
    (fp32r matmuls run LOW_HIGH 2-pass on TRN2; bf16 streams 1
    row/cycle),
  - x sliced to used channels, zero-padded to (H+2)x(W+2), cast bf16.
    The pad border makes every conv tap a plain strided view of the
    flat image -- no edge-column recompute -- and each row-band lands
    as one contiguous DMA per partition.

On-chip ("halves" layout, 2*K_used <= 128): image 0 lives in partitions
[0, K), image 1 in [K, 2K). The two images' matmuls interleave per tap,
so two PE row-group streams run concurrently (~2 cols/cycle, measured
107ns per 512-col matmul pair-member = the structural floor for this
mask: every (cin, tap) row feeds only 64 of 128 couts). Conv = 9
shifted matmuls accumulating in a full PSUM bank per (window of 4 rows,
image); w-outer order lets copy-out chase the stream.

Schedule notes (all measured on the NTFF profiles):
  - The DMA-engine pool is shared (~300-360GB/s/core aggregate); bulk
    transfers starve small ones on other queues. The band WAW-overlap
    rows chain all x transfers into a zipper so band0/wT land by ~11us
    and the first matmul issues then; the PE runs its first ~9us of
    active time under a hardware 0.5-util p-state ramp regardless (warm
    -up matmuls just burn that window -- don't).
  - y leaves the device as bf16 (PSUM is f32; one output rounding costs
    ~2^-9 rel vs the 2e-2 gate) halving output traffic; the host
    upcasts. Copy-out is split: img0 on the scalar engine (activation
    w/ fused bias), img1 on the vector engine (tensor_scalar_add), so
    neither trails the stream. Output chunks split across the two HWDGE
    queues, tapering near the end so the last transfer barely trails
    the final matmul.
"""

import numpy as np
import ml_dtypes
from contextlib import ExitStack

import concourse.bass as bass
import concourse.tile as tile
from concourse import mybir, bacc
from concourse import bass_utils

# Problem shape (hardcoded per contract)
B, CIN, COUT, H, W = 16, 128, 128, 128, 128
KH, KW = 3, 3
BLK = 32
NCORES = 8
BPC = B // NCORES            # images per core

HP, WP = H + 2, W + 2        # host-padded image
LF = HP * WP                 # flat padded image length (16900)
RPW = 4                      # output rows per PSUM window (N=512 = full bank)
# windows: 31 full 4-row windows + two 2-row halves at the end, so the
# last copy-out + DMA overlaps the true final window's compute
WINS = [(RPW * k, RPW) for k in range(31)] + [(124, 2), (126, 2)]
# output chunks (win_index, n_windows): 6-window chunks give large
# packets, tapering to single windows at the end so the last transfer
# barely trails the final matmul
CHUNKS = [(0, 6), (6, 6), (12, 6), (18, 6), (24, 4), (28, 2), (30, 1), (31, 1), (32, 1)]
CHUNK_WIN = 6                # max chunk size (stage tile capacity)

# x row-bands (in padded rows), sized so each lands before the matmul
# stream reaches it. Band 0 rides the scalar HWDGE queue (small, early);
# the bulk goes via SWDGE in large packets, finishing before the output
# stream needs the shared DMA-engine pool.
BAND0A = (0, 6)              # covers window 0; minimal first-wave piece
BAND0B = (6, 6)              # covers window 1
# bulk bands: img0 streams on SWDGE, img1 on the scalar HWDGE queue in
# parallel. Every band overlaps the previous band's last row: the WAW
# dependencies chain the transfers into a cross-queue zipper (b0i0,
# b0i1 | b1i0, b1i1, b2i0, ...), so at most ~two transfers share the
# DMA-engine pool at once and none starves the small early bands.
BANDS = [(11, 30), (40, 50), (89, 41)]

_cache = {}


def _build(K_used, halves):
    """Build + compile the per-core SPMD program.

    K_used: number of surviving input channels (multiple of 32)
    halves: both images packed into one 128-partition buffer at bases
            (0, K_used) for concurrent PE row-group streams
    """
    P = 2 * K_used if halves else K_used
    assert P <= 128

    nc = bacc.Bacc("TRN2", target_bir_lowering=False, debug=False)
    f32 = mybir.dt.float32
    bf16 = mybir.dt.bfloat16

    x_in = nc.dram_tensor("xp", [BPC, K_used, HP, WP], bf16, kind="ExternalInput").ap()
    wt_in = nc.dram_tensor("wt", [P, KH * KW, COUT], bf16, kind="ExternalInput").ap()
    b_in = nc.dram_tensor("bias2d", [COUT, 1], f32, kind="ExternalInput").ap()
    # y leaves the device as bf16 (PSUM accumulates in f32; one rounding on
    # the way out costs ~2^-9 relative -- tolerance is 2e-2) and the host
    # upcasts. This halves the dominant HBM/DMA-pool cost (output traffic).
    y_out = nc.dram_tensor("y", [BPC, COUT, H, W], bf16, kind="ExternalOutput").ap()

    with tile.TileContext(nc) as tc:
        with ExitStack() as ctx:
            singles = ctx.enter_context(tc.tile_pool(name="singles", bufs=1))
            stage_pool = ctx.enter_context(tc.tile_pool(name="ystage", bufs=6))
            psum_pool = ctx.enter_context(
                tc.tile_pool(name="psum", bufs=8, space="PSUM")
            )

            # ---- x buffers -----------------------------------------------
            if halves:
                xb0 = singles.tile([128, LF], bf16, name="xbuf")
                xbufs = [xb0, xb0]
                pbase = [0, K_used]
            else:
                xbufs = [
                    singles.tile([K_used, LF], bf16, name=f"xbuf{i}")
                    for i in range(BPC)
                ]
                pbase = [0] * BPC

            def band_dma(eng, b, r0, nr):
                src = x_in[b].rearrange("c h w -> c (h w)")
                eng.dma_start(
                    out=xbufs[b][pbase[b] : pbase[b] + K_used, r0 * WP : (r0 + nr) * WP],
                    in_=src[:, r0 * WP : (r0 + nr) * WP],
                )

            # sync queue: wT only (gates the first LDWEIGHTS), then img0
            # output chunks. scalar queue: band0 pair + bias, then img1
            # output chunks. Bulk x bands stream on gpsimd SWDGE and are
            # done before the output wants the DMA-engine pool.
            # wT arrives in two pieces: tap 0 alone gates the first
            # LDWEIGHTS; the remaining taps follow and land before window
            # 0's later taps need them. (Finer just-in-time splits of wT
            # and band0 were measured: the earlier stream start is eaten
            # by new arrival-jitter stalls.)
            # First wave = tap-0 weights + 6 rows per image (0.23MB) so the
            # pair stream starts as soon as the pool can deliver it; the
            # rest follows with ~1us deadline margins (per-tap trickling
            # was measured: margins that thin turn into arrival jitter).
            wT = singles.tile([P, KH * KW, COUT], bf16, name="wT")
            nc.sync.dma_start(out=wT[:, 0:1, :], in_=wt_in[:, 0:1, :])
            nc.sync.dma_start(out=wT[:, 1:5, :], in_=wt_in[:, 1:5, :])
            nc.sync.dma_start(out=wT[:, 5:, :], in_=wt_in[:, 5:, :])
            # band0 halves per image on different queues so they load in
            # parallel. The bulk-band WAW zipper is cross-linked: b1i0
            # (gpsimd) waits on b0b_i0 (scalar), b1i1 (scalar) waits on
            # b0b_i1 (gpsimd).
            band_dma(nc.scalar, 0, *BAND0A)
            band_dma(nc.gpsimd, 1, *BAND0A)
            band_dma(nc.scalar, 0, *BAND0B)
            band_dma(nc.gpsimd, 1, *BAND0B)
            bias_sb = singles.tile([COUT, 1], f32, name="bias_sb")
            nc.scalar.dma_start(out=bias_sb, in_=b_in)
            for r0, nr in BANDS:
                band_dma(nc.gpsimd, 0, r0, nr)
            for r0, nr in BANDS:
                band_dma(nc.scalar, 1, r0, nr)

            # ---- main loop: windows of up to RPW output rows --------------
            for c0, cw in CHUNKS:
                wins = list(range(c0, c0 + cw))
                chunk_r0 = WINS[c0][0]
                chunk_nr = sum(WINS[k][1] for k in wins)
                stages = [
                    stage_pool.tile(
                        [COUT, RPW * CHUNK_WIN * W], bf16, tag=f"st{b}", name=f"st{b}_{c0}"
                    )
                    for b in range(BPC)
                ]

                for w in wins:
                    r0, nrows = WINS[w]
                    N = nrows * W
                    ps = [
                        psum_pool.tile([128, 512], f32, tag="ps", name=f"ps{b}_{w}")
                        for b in range(BPC)
                    ]
                    for t in range(KH * KW):
                        dh, dw = divmod(t, 3)
                        off = (r0 + dh) * WP + dw
                        for b in range(BPC):
                            v = xbufs[b][pbase[b] : pbase[b] + K_used, off : off + 1]
                            rhs = bass.AP(
                                tensor=v.tensor,
                                offset=v.offset,
                                ap=[list(v.ap[0]), [WP, nrows], [1, W]],
                            )
                            nc.tensor.matmul(
                                ps[b][:, :N],
                                wT[pbase[b] : pbase[b] + K_used, t, :],
                                rhs,
                                start=(t == 0),
                                stop=(t == KH * KW - 1),
                            )
                    # per-window bias + PSUM->SBUF drain: img0 on the scalar
                    # engine, img1 on the (otherwise idle) vector engine
                    so = (r0 - chunk_r0) * W
                    nc.scalar.activation(
                        out=stages[0][:, so : so + N],
                        in_=ps[0][:, :N],
                        func=mybir.ActivationFunctionType.Identity,
                        bias=bias_sb,
                        scale=1.0,
                    )
                    nc.vector.tensor_scalar_add(
                        out=stages[1][:, so : so + N],
                        in0=ps[1][:, :N],
                        scalar1=bias_sb,
                    )

                # split output across both HWDGE queues (img0: sync, img1:
                # scalar) -- a single queue tops out well under the pool rate
                for b, eng in zip(range(BPC), (nc.sync, nc.scalar)):
                    eng.dma_start(
                        out=y_out[b][:, chunk_r0 : chunk_r0 + chunk_nr, :],
                        in_=stages[b][:, : chunk_nr * W].rearrange(
                            "p (r s) -> p r s", s=W
                        ),
                    )

    nc.compile()
    return nc


def kernel(x, weight, bias, mask):
    x = np.ascontiguousarray(np.asarray(x, dtype=np.float32))
    weight = np.ascontiguousarray(np.asarray(weight, dtype=np.float32))
    bias = np.ascontiguousarray(np.asarray(bias, dtype=np.float32))
    mask = np.ascontiguousarray(np.asarray(mask, dtype=np.float32))
    bf16 = ml_dtypes.bfloat16

    # --- host-side schedule specialization from the runtime mask ----------
    wm = weight * mask
    blk_any = (
        np.abs(wm).reshape(COUT, CIN // BLK, BLK, KH, KW).sum(axis=(0, 2, 3, 4)) > 0
    )
    used_ibs = [ib for ib in range(CIN // BLK) if blk_any[ib]]
    if not used_ibs:
        used_ibs = [0]
    K_used = BLK * len(used_ibs)
    halves = 2 * K_used <= 128

    used_ch = np.concatenate(
        [np.arange(ib * BLK, (ib + 1) * BLK) for ib in used_ibs]
    )

    key = (K_used, halves)
    if key not in _cache:
        _cache[key] = _build(K_used, halves)
    nc = _cache[key]

    # lhsT layout: wt[c, t, o] = (w*m)[o, used_ch[c], t], replicated per
    # partition half so each image's row group has its own copy
    wt = wm[:, used_ch].reshape(COUT, K_used, KH * KW).transpose(1, 2, 0)
    if halves:
        wt = np.concatenate([wt, wt], axis=0)
    wt = np.ascontiguousarray(wt.astype(bf16))
    bias2d = np.ascontiguousarray(bias[:, None])

    in_maps = []
    for core in range(NCORES):
        xs = x[core * BPC : (core + 1) * BPC][:, used_ch]
        xp = np.zeros((BPC, K_used, HP, WP), dtype=bf16)
        xp[:, :, 1 : H + 1, 1 : W + 1] = xs.astype(bf16)
        in_maps.append({"xp": xp, "wt": wt, "bias2d": bias2d})

    global _last_in_maps
    _last_in_maps = in_maps

    res = bass_utils.run_bass_kernel_spmd(nc, in_maps, core_ids=list(range(NCORES)))
    y = np.concatenate(
        [res.results[c]["y"].astype(np.float32) for c in range(NCORES)], axis=0
    )
    return y


_last_in_maps = None


# revision 52
# speedup vs baseline: 1.0259x; 1.0259x over previous
"""Blocksparse conv2d (3x3, stride 1, pad 1) on 8 Trainium2 NeuronCores.

Strategy
--------
Data-parallel over batch: 16 images -> 2 per core, identical SPMD program.

The mask zeroes whole 32x32 (cout, cin) channel blocks; the host inspects
the runtime mask and keeps only the input-channel blocks that survive
(seed-42 mask: channels 64..127, K_used=64). Host-side prep (all cheap):
  - w*mask, slice to used channels, transpose to the PE lhsT layout
    [K_used, 9, COUT], replicate per partition half, cast to bf16
    (fp32r matmuls run LOW_HIGH 2-pass on TRN2; bf16 streams 1
    row/cycle),
  - x sliced to used channels, zero-padded to (H+2)x(W+2), cast bf16.
    The pad border makes every conv tap a plain strided view of the
    flat image -- no edge-column recompute -- and each row-band lands
    as one contiguous DMA per partition.

On-chip ("halves" layout, 2*K_used <= 128): image 0 lives in partitions
[0, K), image 1 in [K, 2K). The two images' matmuls interleave per tap,
so two PE row-group streams run concurrently (~2 cols/cycle, measured
107ns per 512-col matmul pair-member = the structural floor for this
mask: every (cin, tap) row feeds only 64 of 128 couts). Conv = 9
shifted matmuls accumulating in a full PSUM bank per (window of 4 rows,
image); w-outer order lets copy-out chase the stream.

Schedule notes (all measured on the NTFF profiles):
  - The DMA-engine pool is shared (~300-360GB/s/core aggregate); bulk
    transfers starve small ones on other queues. The band WAW-overlap
    rows chain all x transfers into a zipper so band0/wT land by ~11us
    and the first matmul issues then; the PE runs its first ~9us of
    active time under a hardware 0.5-util p-state ramp regardless (warm
    -up matmuls just burn that window -- don't).
  - y leaves the device as bf16 (PSUM is f32; one output rounding costs
    ~2^-9 rel vs the 2e-2 gate) halving output traffic; the host
    upcasts. Copy-out is split: img0 on the scalar engine (activation
    w/ fused bias), img1 on the vector engine (tensor_scalar_add), so
    neither trails the stream. Output chunks split across the two HWDGE
    queues, tapering near the end so the last transfer barely trails
    the final matmul.
"""

import numpy as np
import ml_dtypes
from contextlib import ExitStack

import concourse.bass as bass
import concourse.tile as tile
from concourse import mybir, bacc
from concourse import bass_utils

# Problem shape (hardcoded per contract)
B, CIN, COUT, H, W = 16, 128, 128, 128, 128
KH, KW = 3, 3
BLK = 32
NCORES = 8
BPC = B // NCORES            # images per core

HP, WP = H + 2, W + 2        # host-padded image
LF = HP * WP                 # flat padded image length (16900)
RPW = 4                      # output rows per PSUM window (N=512 = full bank)
# windows: 31 full 4-row windows + two 2-row halves at the end, so the
# last copy-out + DMA overlaps the true final window's compute
WINS = [(RPW * k, RPW) for k in range(31)] + [(124, 2), (126, 2)]
# output chunks (win_index, n_windows): 6-window chunks give large
# packets, tapering to single windows at the end so the last transfer
# barely trails the final matmul
CHUNKS = [(0, 6), (6, 6), (12, 6), (18, 6), (24, 4), (28, 2), (30, 1), (31, 1), (32, 1)]
CHUNK_WIN = 6                # max chunk size (stage tile capacity)

# x row-bands (in padded rows), sized so each lands before the matmul
# stream reaches it. Band 0 rides the scalar HWDGE queue (small, early);
# the bulk goes via SWDGE in large packets, finishing before the output
# stream needs the shared DMA-engine pool.
BAND0 = (0, 12)              # covers windows 0-1; small for a fast start
# bulk bands: img0 streams on SWDGE, img1 on the scalar HWDGE queue in
# parallel. Every band overlaps the previous band's last row: the WAW
# dependencies chain the transfers into a cross-queue zipper (b0i0,
# b0i1 | b1i0, b1i1, b2i0, ...), so at most ~two transfers share the
# DMA-engine pool at once and none starves the small early bands.
BANDS = [(11, 30), (40, 50), (89, 41)]

_cache = {}


def _build(K_used, halves):
    """Build + compile the per-core SPMD program.

    K_used: number of surviving input channels (multiple of 32)
    halves: both images packed into one 128-partition buffer at bases
            (0, K_used) for concurrent PE row-group streams
    """
    P = 2 * K_used if halves else K_used
    assert P <= 128

    nc = bacc.Bacc("TRN2", target_bir_lowering=False, debug=False)
    f32 = mybir.dt.float32
    bf16 = mybir.dt.bfloat16

    x_in = nc.dram_tensor("xp", [BPC, K_used, HP, WP], bf16, kind="ExternalInput").ap()
    wt_in = nc.dram_tensor("wt", [P, KH * KW, COUT], bf16, kind="ExternalInput").ap()
    b_in = nc.dram_tensor("bias2d", [COUT, 1], f32, kind="ExternalInput").ap()
    # y leaves the device as bf16 (PSUM accumulates in f32; one rounding on
    # the way out costs ~2^-9 relative -- tolerance is 2e-2) and the host
    # upcasts. This halves the dominant HBM/DMA-pool cost (output traffic).
    y_out = nc.dram_tensor("y", [BPC, COUT, H, W], bf16, kind="ExternalOutput").ap()

    with tile.TileContext(nc) as tc:
        with ExitStack() as ctx:
            singles = ctx.enter_context(tc.tile_pool(name="singles", bufs=1))
            stage_pool = ctx.enter_context(tc.tile_pool(name="ystage", bufs=6))
            psum_pool = ctx.enter_context(
                tc.tile_pool(name="psum", bufs=8, space="PSUM")
            )

            # ---- x buffers -----------------------------------------------
            if halves:
                xb0 = singles.tile([128, LF], bf16, name="xbuf")
                xbufs = [xb0, xb0]
                pbase = [0, K_used]
            else:
                xbufs = [
                    singles.tile([K_used, LF], bf16, name=f"xbuf{i}")
                    for i in range(BPC)
                ]
                pbase = [0] * BPC

            def band_dma(eng, b, r0, nr):
                src = x_in[b].rearrange("c h w -> c (h w)")
                eng.dma_start(
                    out=xbufs[b][pbase[b] : pbase[b] + K_used, r0 * WP : (r0 + nr) * WP],
                    in_=src[:, r0 * WP : (r0 + nr) * WP],
                )

            # sync queue: wT only (gates the first LDWEIGHTS), then img0
            # output chunks. scalar queue: band0 pair + bias, then img1
            # output chunks. Bulk x bands stream on gpsimd SWDGE and are
            # done before the output wants the DMA-engine pool.
            # wT arrives in two pieces: tap 0 alone gates the first
            # LDWEIGHTS; the remaining taps follow and land before window
            # 0's later taps need them. (Finer just-in-time splits of wT
            # and band0 were measured: the earlier stream start is eaten
            # by new arrival-jitter stalls.)
            # wT arrives in two pieces: tap 0 alone gates the first
            # LDWEIGHTS; the remaining taps land before window 0 needs
            # them. (Finer just-in-time splits of wT/band0 were measured
            # twice: the earlier stream start converts 1:1 into mid-stream
            # arrival-jitter stalls -- startup is delivery-latency-bound.)
            wT = singles.tile([P, KH * KW, COUT], bf16, name="wT")
            nc.sync.dma_start(out=wT[:, 0:1, :], in_=wt_in[:, 0:1, :])
            nc.sync.dma_start(out=wT[:, 1:, :], in_=wt_in[:, 1:, :])
            # band0 halves go to different queues so they load in parallel
            # (serialized on one queue, img1's half gated the first matmul
            # pair ~1.3us late). The bulk-band WAW zipper is cross-linked:
            # b1i0 (gpsimd) waits on b0i0 (scalar), b1i1 (scalar) waits on
            # b0i1 (gpsimd).
            band_dma(nc.scalar, 0, *BAND0)
            band_dma(nc.gpsimd, 1, *BAND0)
            bias_sb = singles.tile([COUT, 1], f32, name="bias_sb")
            nc.scalar.dma_start(out=bias_sb, in_=b_in)
            for r0, nr in BANDS:
                band_dma(nc.gpsimd, 0, r0, nr)
            for r0, nr in BANDS:
                band_dma(nc.scalar, 1, r0, nr)

            # ---- main loop: windows of up to RPW output rows --------------
            for c0, cw in CHUNKS:
                wins = list(range(c0, c0 + cw))
                chunk_r0 = WINS[c0][0]
                chunk_nr = sum(WINS[k][1] for k in wins)
                stages = [
                    stage_pool.tile(
                        [COUT, RPW * CHUNK_WIN * W], bf16, tag=f"st{b}", name=f"st{b}_{c0}"
                    )
                    for b in range(BPC)
                ]

                for w in wins:
                    r0, nrows = WINS[w]
                    N = nrows * W
                    ps = [
                        psum_pool.tile([128, 512], f32, tag="ps", name=f"ps{b}_{w}")
                        for b in range(BPC)
                    ]
                    for t in range(KH * KW):
                        dh, dw = divmod(t, 3)
                        off = (r0 + dh) * WP + dw
                        for b in range(BPC):
                            v = xbufs[b][pbase[b] : pbase[b] + K_used, off : off + 1]
                            rhs = bass.AP(
                                tensor=v.tensor,
                                offset=v.offset,
                                ap=[list(v.ap[0]), [WP, nrows], [1, W]],
                            )
                            nc.tensor.matmul(
                                ps[b][:, :N],
                                wT[pbase[b] : pbase[b] + K_used, t, :],
                                rhs,
                                start=(t == 0),
                                stop=(t == KH * KW - 1),
                            )
                    # per-window bias + PSUM->SBUF drain: img0 on the scalar
                    # engine, img1 on the (otherwise idle) vector engine
                    so = (r0 - chunk_r0) * W
                    nc.scalar.activation(
                        out=stages[0][:, so : so + N],
                        in_=ps[0][:, :N],
                        func=mybir.ActivationFunctionType.Identity,
                        bias=bias_sb,
                        scale=1.0,
                    )
                    nc.vector.tensor_scalar_add(
                        out=stages[1][:, so : so + N],
                        in0=ps[1][:, :N],
                        scalar1=bias_sb,
                    )

                # split output across both HWDGE queues (img0: sync, img1:
                # scalar) -- a single queue tops out well under the pool rate
                for b, eng in zip(range(BPC), (nc.sync, nc.scalar)):
                    eng.dma_start(
                        out=y_out[b][:, chunk_r0 : chunk_r0 + chunk_nr, :],
                        in_=stages[b][:, : chunk_nr * W].rearrange(
                            "p (r s) -> p r s", s=W
                        ),
                    )

    nc.compile()
    return nc


def kernel(x, weight, bias, mask):
    x = np.ascontiguousarray(np.asarray(x, dtype=np.float32))
    weight = np.ascontiguousarray(np.asarray(weight, dtype=np.float32))
    bias = np.ascontiguousarray(np.asarray(bias, dtype=np.float32))
    mask = np.ascontiguousarray(np.asarray(mask, dtype=np.float32))
    bf16 = ml_dtypes.bfloat16

    # --- host-side schedule specialization from the runtime mask ----------
    wm = weight * mask
    blk_any = (
        np.abs(wm).reshape(COUT, CIN // BLK, BLK, KH, KW).sum(axis=(0, 2, 3, 4)) > 0
    )
    used_ibs = [ib for ib in range(CIN // BLK) if blk_any[ib]]
    if not used_ibs:
        used_ibs = [0]
    K_used = BLK * len(used_ibs)
    halves = 2 * K_used <= 128

    used_ch = np.concatenate(
        [np.arange(ib * BLK, (ib + 1) * BLK) for ib in used_ibs]
    )

    key = (K_used, halves)
    if key not in _cache:
        _cache[key] = _build(K_used, halves)
    nc = _cache[key]

    # lhsT layout: wt[c, t, o] = (w*m)[o, used_ch[c], t], replicated per
    # partition half so each image's row group has its own copy
    wt = wm[:, used_ch].reshape(COUT, K_used, KH * KW).transpose(1, 2, 0)
    if halves:
        wt = np.concatenate([wt, wt], axis=0)
    wt = np.ascontiguousarray(wt.astype(bf16))
    bias2d = np.ascontiguousarray(bias[:, None])

    in_maps = []
    for core in range(NCORES):
        xs = x[core * BPC : (core + 1) * BPC][:, used_ch]
        xp = np.zeros((BPC, K_used, HP, WP), dtype=bf16)
        xp[:, :, 1 : H + 1, 1 : W + 1] = xs.astype(bf16)
        in_maps.append({"xp": xp, "wt": wt, "bias2d": bias2d})

    global _last_in_maps
    _last_in_maps = in_maps

    res = bass_utils.run_bass_kernel_spmd(nc, in_maps, core_ids=list(range(NCORES)))
    y = np.concatenate(
        [res.results[c]["y"].astype(np.float32) for c in range(NCORES)], axis=0
    )
    return y


_last_in_maps = None


# revision 53
# speedup vs baseline: 1.0267x; 1.0008x over previous
"""Blocksparse conv2d (3x3, stride 1, pad 1) on 8 Trainium2 NeuronCores.

Strategy
--------
Data-parallel over batch: 16 images -> 2 per core, identical SPMD program.

The mask zeroes whole 32x32 (cout, cin) channel blocks; the host inspects
the runtime mask and keeps only the input-channel blocks that survive
(seed-42 mask: channels 64..127, K_used=64). Host-side prep (all cheap):
  - w*mask, slice to used channels, transpose to the PE lhsT layout
    [K_used, 9, COUT], replicate per partition half, cast to bf16
    (fp32r matmuls run LOW_HIGH 2-pass on TRN2; bf16 streams 1
    row/cycle),
  - x sliced to used channels, zero-padded to (H+2)x(W+2), cast bf16.
    The pad border makes every conv tap a plain strided view of the
    flat image -- no edge-column recompute -- and each row-band lands
    as one contiguous DMA per partition.

On-chip ("halves" layout, 2*K_used <= 128): image 0 lives in partitions
[0, K), image 1 in [K, 2K). The two images' matmuls interleave per tap,
so two PE row-group streams run concurrently (~2 cols/cycle, measured
107ns per 512-col matmul pair-member = the structural floor for this
mask: every (cin, tap) row feeds only 64 of 128 couts). Conv = 9
shifted matmuls accumulating in a full PSUM bank per (window of 4 rows,
image); w-outer order lets copy-out chase the stream.

Schedule notes (all measured on the NTFF profiles):
  - The DMA-engine pool is shared (~300-360GB/s/core aggregate); bulk
    transfers starve small ones on other queues. The band WAW-overlap
    rows chain all x transfers into a zipper so band0/wT land by ~11us
    and the first matmul issues then; the PE runs its first ~9us of
    active time under a hardware 0.5-util p-state ramp regardless (warm
    -up matmuls just burn that window -- don't).
  - y leaves the device as bf16 (PSUM is f32; one output rounding costs
    ~2^-9 rel vs the 2e-2 gate) halving output traffic; the host
    upcasts. Copy-out is split: img0 on the scalar engine (activation
    w/ fused bias), img1 on the vector engine (tensor_scalar_add), so
    neither trails the stream. Output chunks split across the two HWDGE
    queues, tapering near the end so the last transfer barely trails
    the final matmul.
"""

import numpy as np
import ml_dtypes
from contextlib import ExitStack

import concourse.bass as bass
import concourse.tile as tile
from concourse import mybir, bacc
from concourse import bass_utils

# Problem shape (hardcoded per contract)
B, CIN, COUT, H, W = 16, 128, 128, 128, 128
KH, KW = 3, 3
BLK = 32
NCORES = 8
BPC = B // NCORES            # images per core

HP, WP = H + 2, W + 2        # host-padded image
LF = HP * WP                 # flat padded image length (16900)
RPW = 4                      # output rows per PSUM window (N=512 = full bank)
# windows: 31 full 4-row windows + two 2-row halves at the end, so the
# last copy-out + DMA overlaps the true final window's compute
WINS = [(RPW * k, RPW) for k in range(31)] + [(124, 2), (126, 2)]
# output chunks (win_index, n_windows): 6-window chunks give large
# packets, tapering to single windows at the end so the last transfer
# barely trails the final matmul
CHUNKS = [(0, 6), (6, 6), (12, 6), (18, 6), (24, 4), (28, 2), (30, 1), (31, 1), (32, 1)]
CHUNK_WIN = 6                # max chunk size (stage tile capacity)

# x row-bands (in padded rows), sized so each lands before the matmul
# stream reaches it. Band 0 rides the scalar HWDGE queue (small, early);
# the bulk goes via SWDGE in large packets, finishing before the output
# stream needs the shared DMA-engine pool.
BAND0 = (0, 12)              # covers windows 0-1; small for a fast start
# bulk bands: img0 streams on SWDGE, img1 on the scalar HWDGE queue in
# parallel. Every band overlaps the previous band's last row: the WAW
# dependencies chain the transfers into a cross-queue zipper (b0i0,
# b0i1 | b1i0, b1i1, b2i0, ...), so at most ~two transfers share the
# DMA-engine pool at once and none starves the small early bands.
BANDS = [(11, 30), (40, 50), (89, 41)]

_cache = {}


def _build(K_used, halves):
    """Build + compile the per-core SPMD program.

    K_used: number of surviving input channels (multiple of 32)
    halves: both images packed into one 128-partition buffer at bases
            (0, K_used) for concurrent PE row-group streams
    """
    P = 2 * K_used if halves else K_used
    assert P <= 128

    nc = bacc.Bacc("TRN2", target_bir_lowering=False, debug=False)
    f32 = mybir.dt.float32
    bf16 = mybir.dt.bfloat16

    x_in = nc.dram_tensor("xp", [BPC, K_used, HP, WP], bf16, kind="ExternalInput").ap()
    wt_in = nc.dram_tensor("wt", [P, KH * KW, COUT], bf16, kind="ExternalInput").ap()
    b_in = nc.dram_tensor("bias2d", [COUT, 1], f32, kind="ExternalInput").ap()
    # y leaves the device as bf16 (PSUM accumulates in f32; one rounding on
    # the way out costs ~2^-9 relative -- tolerance is 2e-2) and the host
    # upcasts. This halves the dominant HBM/DMA-pool cost (output traffic).
    y_out = nc.dram_tensor("y", [BPC, COUT, H, W], bf16, kind="ExternalOutput").ap()

    with tile.TileContext(nc) as tc:
        with ExitStack() as ctx:
            singles = ctx.enter_context(tc.tile_pool(name="singles", bufs=1))
            stage_pool = ctx.enter_context(tc.tile_pool(name="ystage", bufs=6))
            psum_pool = ctx.enter_context(
                tc.tile_pool(name="psum", bufs=8, space="PSUM")
            )

            # ---- x buffers -----------------------------------------------
            if halves:
                xb0 = singles.tile([128, LF], bf16, name="xbuf")
                xbufs = [xb0, xb0]
                pbase = [0, K_used]
            else:
                xbufs = [
                    singles.tile([K_used, LF], bf16, name=f"xbuf{i}")
                    for i in range(BPC)
                ]
                pbase = [0] * BPC

            def band_dma(eng, b, r0, nr):
                src = x_in[b].rearrange("c h w -> c (h w)")
                eng.dma_start(
                    out=xbufs[b][pbase[b] : pbase[b] + K_used, r0 * WP : (r0 + nr) * WP],
                    in_=src[:, r0 * WP : (r0 + nr) * WP],
                )

            # sync queue: wT only (gates the first LDWEIGHTS), then img0
            # output chunks. scalar queue: band0 pair + bias, then img1
            # output chunks. Bulk x bands stream on gpsimd SWDGE and are
            # done before the output wants the DMA-engine pool.
            # wT arrives in two pieces: tap 0 alone gates the first
            # LDWEIGHTS; the remaining taps follow and land before window
            # 0's later taps need them. (Finer just-in-time splits of wT
            # and band0 were measured: the earlier stream start is eaten
            # by new arrival-jitter stalls.)
            # wT arrives in two pieces: tap 0 alone gates the first
            # LDWEIGHTS; the remaining taps land before window 0 needs
            # them. (Finer just-in-time splits of wT/band0 were measured
            # twice: the earlier stream start converts 1:1 into mid-stream
            # arrival-jitter stalls -- startup is delivery-latency-bound.)
            wT = singles.tile([P, KH * KW, COUT], bf16, name="wT")
            nc.sync.dma_start(out=wT[:, 0:1, :], in_=wt_in[:, 0:1, :])
            nc.sync.dma_start(out=wT[:, 1:, :], in_=wt_in[:, 1:, :])
            # band0 halves go to different queues so they load in parallel
            # (serialized on one queue, img1's half gated the first matmul
            # pair ~1.3us late). The bulk-band WAW zipper is cross-linked:
            # b1i0 (gpsimd) waits on b0i0 (scalar), b1i1 (scalar) waits on
            # b0i1 (gpsimd).
            band_dma(nc.scalar, 0, *BAND0)
            band_dma(nc.gpsimd, 1, *BAND0)
            # bias rides the sync queue (lands ~11.5us, first ACTIVATE needs
            # it ~14us) so img1's first bulk band issues one slot earlier on
            # the scalar queue -- its late arrival was the last recurring
            # stream gap (w2-img1 at ~18us).
            bias_sb = singles.tile([COUT, 1], f32, name="bias_sb")
            nc.sync.dma_start(out=bias_sb, in_=b_in)
            for r0, nr in BANDS:
                band_dma(nc.gpsimd, 0, r0, nr)
            for r0, nr in BANDS:
                band_dma(nc.scalar, 1, r0, nr)

            # ---- main loop: windows of up to RPW output rows --------------
            for c0, cw in CHUNKS:
                wins = list(range(c0, c0 + cw))
                chunk_r0 = WINS[c0][0]
                chunk_nr = sum(WINS[k][1] for k in wins)
                stages = [
                    stage_pool.tile(
                        [COUT, RPW * CHUNK_WIN * W], bf16, tag=f"st{b}", name=f"st{b}_{c0}"
                    )
                    for b in range(BPC)
                ]

                for w in wins:
                    r0, nrows = WINS[w]
                    N = nrows * W
                    ps = [
                        psum_pool.tile([128, 512], f32, tag="ps", name=f"ps{b}_{w}")
                        for b in range(BPC)
                    ]
                    for t in range(KH * KW):
                        dh, dw = divmod(t, 3)
                        off = (r0 + dh) * WP + dw
                        for b in range(BPC):
                            v = xbufs[b][pbase[b] : pbase[b] + K_used, off : off + 1]
                            rhs = bass.AP(
                                tensor=v.tensor,
                                offset=v.offset,
                                ap=[list(v.ap[0]), [WP, nrows], [1, W]],
                            )
                            nc.tensor.matmul(
                                ps[b][:, :N],
                                wT[pbase[b] : pbase[b] + K_used, t, :],
                                rhs,
                                start=(t == 0),
                                stop=(t == KH * KW - 1),
                            )
                    # per-window bias + PSUM->SBUF drain: img0 on the scalar
                    # engine, img1 on the (otherwise idle) vector engine
                    so = (r0 - chunk_r0) * W
                    nc.scalar.activation(
                        out=stages[0][:, so : so + N],
                        in_=ps[0][:, :N],
                        func=mybir.ActivationFunctionType.Identity,
                        bias=bias_sb,
                        scale=1.0,
                    )
                    nc.vector.tensor_scalar_add(
                        out=stages[1][:, so : so + N],
                        in0=ps[1][:, :N],
                        scalar1=bias_sb,
                    )

                # split output across both HWDGE queues (img0: sync, img1:
                # scalar) -- a single queue tops out well under the pool rate
                for b, eng in zip(range(BPC), (nc.sync, nc.scalar)):
                    eng.dma_start(
                        out=y_out[b][:, chunk_r0 : chunk_r0 + chunk_nr, :],
                        in_=stages[b][:, : chunk_nr * W].rearrange(
                            "p (r s) -> p r s", s=W
                        ),
                    )

    nc.compile()
    return nc


def kernel(x, weight, bias, mask):
    x = np.ascontiguousarray(np.asarray(x, dtype=np.float32))
    weight = np.ascontiguousarray(np.asarray(weight, dtype=np.float32))
    bias = np.ascontiguousarray(np.asarray(bias, dtype=np.float32))
    mask = np.ascontiguousarray(np.asarray(mask, dtype=np.float32))
    bf16 = ml_dtypes.bfloat16

    # --- host-side schedule specialization from the runtime mask ----------
    wm = weight * mask
    blk_any = (
        np.abs(wm).reshape(COUT, CIN // BLK, BLK, KH, KW).sum(axis=(0, 2, 3, 4)) > 0
    )
    used_ibs = [ib for ib in range(CIN // BLK) if blk_any[ib]]
    if not used_ibs:
        used_ibs = [0]
    K_used = BLK * len(used_ibs)
    halves = 2 * K_used <= 128

    used_ch = np.concatenate(
        [np.arange(ib * BLK, (ib + 1) * BLK) for ib in used_ibs]
    )

    key = (K_used, halves)
    if key not in _cache:
        _cache[key] = _build(K_used, halves)
    nc = _cache[key]

    # lhsT layout: wt[c, t, o] = (w*m)[o, used_ch[c], t], replicated per
    # partition half so each image's row group has its own copy
    wt = wm[:, used_ch].reshape(COUT, K_used, KH * KW).transpose(1, 2, 0)
    if halves:
        wt = np.concatenate([wt, wt], axis=0)
    wt = np.ascontiguousarray(wt.astype(bf16))
    bias2d = np.ascontiguousarray(bias[:, None])

    in_maps = []
    for core in range(NCORES):
        xs = x[core * BPC : (core + 1) * BPC][:, used_ch]
        xp = np.zeros((BPC, K_used, HP, WP), dtype=bf16)
        xp[:, :, 1 : H + 1, 1 : W + 1] = xs.astype(bf16)
        in_maps.append({"xp": xp, "wt": wt, "bias2d": bias2d})

    global _last_in_maps
    _last_in_maps = in_maps

    res = bass_utils.run_bass_kernel_spmd(nc, in_maps, core_ids=list(range(NCORES)))
    y = np.concatenate(
        [res.results[c]["y"].astype(np.float32) for c in range(NCORES)], axis=0
    )
    return y


_last_in_maps = None
